# revision 25
# baseline (speedup 1.0000x reference)
"""Trainium2 Bass kernel for nn_CenterLossN (center-loss style reduction).

Math (per batch n, class c; H=W=384, C=11, N=32):
    res[n,c]   = x[n,c]^2 + centers[n,c]^2 - 2 * x[n,c] @ centers[n,c]
    out[n,h,w] = max_c softmax_c(res)[n,c,h,w] = 1 / sum_c exp(res_c - max_c res_c)
    loss       = sum(clip(out * labels, 1e-12, 1e12)) / (N*H*W)

Device strategy (data-parallel over N across 8 cores, 4 batches/core):
  All matmul inputs fp8e4m3 with DoubleRow perf mode (0.5 cyc/col).
  Host ships, per batch n, partition-major images:
    sta[mc] [128, 34, 128]: slots 3c+kc = (-2x)^T k-block kc of class c,
        columns restricted to row-chunk mc (weights [k, h]); slot 33 =
        eye(128). Split per-mc so the first chunk's weights arrive fast.
    mov_a [128, 44, 384]: per class [cc_k0, cc_k1, cc_k2, ee_mc0].
    mov_b [128, 33, 384]: per class [cc_k2(dup), ee_mc1, ee_mc2].
        (cc = centers moving [k, w]; ee = x^2 + c^2 moving [h, w]; the
        k2 duplicate keeps the DoubleRow (cc_k2, ee_mc) ifmap pair
        inside one tile for mc=1,2.)
    lab [128, 3, 384] bf16.
  Per (n, c, mc): DR1 contracts k-blocks {0,1}; DR2 pairs k-block 2 with
  an identity column injecting ee into the same PSUM group.
  PSUM: two 3-class packs + one 2-class pack per chunk ([128, k, 512]
  f32, bank-aligned slots); one ACT copy drains each pack f32->bf16.

  The per-(n,mc) tail is software-pipelined 4 deep across chunk
  iterations so every op's inputs are produced in an EARLIER iteration:
    iter i: ACT exp(i-2) | PE matmuls(i) + ACT drains(i)
          | DVE max+sub(i-1) | DVE sum+recip+acc(i-3)
  Reciprocal = u16 bit-trick seed (0x7EF3 - bits) + one Newton step;
  the final scalar_tensor_tensor's scalar (-1.003458) undoes the Newton
  sign and corrects the seed's mean bias (value cross-checked against a
  full numpy simulation of the pipeline: -1.003501).
  clip: only label==0 hits the 1e-12 floor (1/sum in [1/11, 1]); host
  adds 1e-12 * count(labels==0).

HW notes from bring-up on this deployment: AluOpType.divide, Ln, ACT
scale!=1.0, Reciprocal-on-ACT (table reload per op), tensor_tensor_reduce,
custom-DVE and GpSimd tensor ops all fail or are too slow; InstReciprocal
runs at ~6.6 ns/elem regardless of dtype (no fast mode).
"""

import numpy as np
import ml_dtypes

N, C, H, W = 32, 11, 384, 384
N_CORES = 8
N_LOC = N // N_CORES          # 4 batches per core
MC = H // 128                 # 3 row-chunks
KC = W // 128                 # 3 contraction chunks
NSTA = C * KC + 1             # 34 weight slots per mc (last = identity)
NMOVA = C * 4                 # 44 slots: cc_k0,cc_k1,cc_k2,ee0 per class
NMOVB = C * 3                 # 33 slots: cc_k2,ee1,ee2 per class
NCH = N_LOC * MC              # 12 chunks per core
MAGIC = 0x7EF3                # bf16 reciprocal seed constant
BIAS_FIX = -1.003458          # Newton sign + seed-bias correction

_BF16 = ml_dtypes.bfloat16
_FP8 = ml_dtypes.float8_e4m3
_COMPILED = None


def _build(n_loc=N_LOC):
    from contextlib import ExitStack
    import concourse.bass as bass
    import concourse.bacc as bacc
    import concourse.tile as tile
    from concourse import mybir

    bf16 = mybir.dt.bfloat16
    f32 = mybir.dt.float32
    fp8 = mybir.dt.float8e4
    u16 = mybir.dt.uint16
    AF = mybir.ActivationFunctionType
    ALU = mybir.AluOpType
    DR = mybir.MatmulPerfMode.DoubleRow

    nc = bacc.Bacc("TRN2", target_bir_lowering=False, debug=False)

    sta_d = nc.dram_tensor("sta", [n_loc, MC, 128, NSTA * 128], fp8,
                           kind="ExternalInput")
    mva_d = nc.dram_tensor("mva", [n_loc, 128, NMOVA * W], fp8,
                           kind="ExternalInput")
    mvb_d = nc.dram_tensor("mvb", [n_loc, 128, NMOVB * W], fp8,
                           kind="ExternalInput")
    lab_d = nc.dram_tensor("lab", [n_loc, 128, MC * W], bf16, kind="ExternalInput")
    cst_d = nc.dram_tensor("cst", [128, 2, W], u16, kind="ExternalInput")
    out_d = nc.dram_tensor("out", [128, 1], f32, kind="ExternalOutput")

    with ExitStack() as ctx:
        tc = ctx.enter_context(tile.TileContext(nc))
        loads = ctx.enter_context(tc.tile_pool(name="loads", bufs=2))
        spool = ctx.enter_context(tc.tile_pool(name="spool", bufs=3))
        dpool = ctx.enter_context(tc.tile_pool(name="dpool", bufs=3))
        tree = ctx.enter_context(tc.tile_pool(name="tree", bufs=2))
        small = ctx.enter_context(tc.tile_pool(name="small", bufs=3))
        singles = ctx.enter_context(tc.tile_pool(name="singles", bufs=1))
        ps3 = ctx.enter_context(tc.tile_pool(name="ps3", bufs=2, space="PSUM"))
        ps2 = ctx.enter_context(tc.tile_pool(name="ps2", bufs=1, space="PSUM"))

        partial = singles.tile([128, NCH + 1], f32)
        magic_t = singles.tile([128, W], u16)
        nc.sync.dma_start(magic_t[:], cst_d[:, 0, :])

        sta_ts, mva_ts, mvb_ts, lab_ts = {}, {}, {}, {}

        def load_first(n):
            # only what chunk (n, 0) needs: weights for mc=0 + mov_a
            sta_ts[(n, 0)] = loads.tile([128, NSTA, 128], fp8, tag="sta0",
                                        name=f"sta_{n}_0")
            nc.sync.dma_start(
                sta_ts[(n, 0)][:],
                sta_d[n, 0].rearrange("p (s w) -> p s w", s=NSTA),
            )
            mva_ts[n] = loads.tile([128, NMOVA, W], fp8, tag="mva", name=f"mva_{n}")
            mva_ap = mva_d[n].rearrange("p (s w) -> p s w", s=NMOVA)
            # split so the first 3-class matmul pack starts after ~1/4 of
            # the transfer (pipeline fill)
            for lo, hi in ((0, 12), (12, 24), (24, 36), (36, NMOVA)):
                nc.sync.dma_start(mva_ts[n][:, lo:hi, :], mva_ap[:, lo:hi, :])

        def load_rest(n):
            def load_sta(mc):
                sta_ts[(n, mc)] = loads.tile([128, NSTA, 128], fp8,
                                             tag=f"sta{mc}", name=f"sta_{n}_{mc}")
                nc.sync.dma_start(
                    sta_ts[(n, mc)][:],
                    sta_d[n, mc].rearrange("p (s w) -> p s w", s=NSTA),
                )

            load_sta(1)
            mvb_ts[n] = loads.tile([128, NMOVB, W], fp8, tag="mvb", name=f"mvb_{n}")
            nc.sync.dma_start(
                mvb_ts[n][:], mvb_d[n].rearrange("p (s w) -> p s w", s=NMOVB)
            )
            load_sta(2)
            lab_ts[n] = loads.tile([128, MC, W], bf16, tag="lab", name=f"lab_{n}")
            nc.gpsimd.dma_start(
                lab_ts[n][:], lab_d[n].rearrange("p (s w) -> p s w", s=MC)
            )

        load_first(0)
        # PE p-state warm-up on the magic tile while the first loads land
        magic8 = magic_t[:].bitcast(fp8)
        pw = ps2.tile([128, 2, 512], f32, tag="p2", name="pwarm")
        for _ in range(12):
            nc.tensor.matmul(
                pw[:, 0, 0:512], magic8[:, 0:128], magic8[:, 0:512],
                start=True, stop=True,
            )
        load_rest(0)

        S_t, D_t = {}, {}

        def stage_mm(i):
            n, mc = i // MC, i % MC
            sta_t = sta_ts[(n, mc)]
            S = spool.tile([128, C, W], bf16, tag="S", name=f"S_{i}")
            S_t[i] = S

            def class_mms(c, out_ap):
                nc.tensor.matmul(
                    out_ap,
                    sta_t[:, 3 * c : 3 * c + 2, :],
                    mva_ts[n][:, 4 * c : 4 * c + 2, :],
                    start=True, stop=False, perf_mode=DR,
                )
                wk = sta_t[:, 3 * c + 2, :]
                w_ap = bass.AP(
                    tensor=wk.tensor, offset=wk.offset,
                    ap=[list(wk.ap[0]), [(NSTA - 1 - (3 * c + 2)) * 128, 2],
                        list(wk.ap[1])],
                )
                if mc == 0:
                    m_ap = mva_ts[n][:, 4 * c + 2 : 4 * c + 4, :]
                elif mc == 1:
                    m_ap = mvb_ts[n][:, 3 * c : 3 * c + 2, :]
                else:
                    mv = mvb_ts[n][:, 3 * c, :]
                    m_ap = bass.AP(
                        tensor=mv.tensor, offset=mv.offset,
                        ap=[list(mv.ap[0]), [2 * W, 2], list(mv.ap[1])],
                    )
                nc.tensor.matmul(
                    out_ap, w_ap, m_ap, start=False, stop=True, perf_mode=DR,
                )

            # three 3-class packs + one 2-class pack per chunk
            for g, (lo, k) in enumerate([(0, 3), (3, 3), (6, 3), (9, 2)]):
                if k == 3:
                    pp = ps3.tile([128, 3, 512], f32, tag="pp", name=f"pp_{i}_{g}")
                else:
                    pp = ps2.tile([128, 2, 512], f32, tag="p2", name=f"p2_{i}")
                for j in range(k):
                    class_mms(lo + j, pp[:, j, 0:W])
                nc.scalar.copy(S[:, lo : lo + k, :], pp[:, 0:k, 0:W])

        def stage_maxsub(i, lo=0, hi=W, sfx=""):
            ww = hi - lo
            S = S_t[i]
            # 4-op max tree: the overlapping first level double-counts
            # plane 5, which is harmless for max: m6[j] = max(S[j], S[j+5])
            m6 = tree.tile([128, 6, ww], bf16, tag="m5" + sfx, name=f"m5{sfx}_{i}")
            nc.vector.tensor_max(m6[:], S[:, 0:6, lo:hi], S[:, 5:11, lo:hi])
            m3 = tree.tile([128, 3, ww], bf16, tag="m2" + sfx, name=f"m2{sfx}_{i}")
            nc.vector.tensor_max(m3[:], m6[:, 0:3, :], m6[:, 3:6, :])
            m = small.tile([128, ww], bf16, tag="m" + sfx, name=f"m{sfx}_{i}")
            nc.vector.tensor_max(m[:], m3[:, 0, :], m3[:, 1, :])
            nc.vector.tensor_max(m[:], m[:], m3[:, 2, :])

            if i not in D_t:
                # 12-plane D: plane 11 DMA-zeroed so the sum tree is 4 ops
                D_t[i] = dpool.tile([128, C + 1, W], bf16, tag="D", name=f"D_{i}")
                nc.gpsimd.dma_start(
                    D_t[i][:, C, :].bitcast(u16), cst_d[:, 1, :]
                )
            D = D_t[i]
            m_ap = m[:]
            m_b = bass.AP(
                tensor=m_ap.tensor, offset=m_ap.offset,
                ap=[list(m_ap.ap[0]), [0, C], list(m_ap.ap[1])],
            )
            nc.vector.tensor_sub(D[:, 0:C, lo:hi], S[:, :, lo:hi], m_b)

        def stage_exp(i, lo=0, hi=W):
            nc.scalar.activation(
                D_t[i][:, 0:C, lo:hi], D_t[i][:, 0:C, lo:hi], AF.Exp
            )

        def stage_sum(i, lo=0, hi=W, sfx="", pslot=None):
            n, mc = i // MC, i % MC
            ww = hi - lo
            pslot = i if pslot is None else pslot
            D = D_t[i]
            a6 = tree.tile([128, 6, ww], bf16, tag="a5" + sfx, name=f"a5{sfx}_{i}")
            nc.vector.tensor_add(a6[:], D[:, 0:6, lo:hi], D[:, 6:12, lo:hi])
            a3 = tree.tile([128, 3, ww], bf16, tag="a2" + sfx, name=f"a2{sfx}_{i}")
            nc.vector.tensor_add(a3[:], a6[:, 0:3, :], a6[:, 3:6, :])
            acc = small.tile([128, ww], bf16, tag="acc" + sfx, name=f"acc{sfx}_{i}")
            nc.vector.tensor_add(acc[:], a3[:, 0, :], a3[:, 1, :])
            nc.vector.tensor_add(acc[:], acc[:], a3[:, 2, :])

            # reciprocal: u16 bit-trick seed + one Newton step.
            r0 = small.tile([128, ww], bf16, tag="r0" + sfx, name=f"r0{sfx}_{i}")
            nc.vector.tensor_sub(
                r0[:].bitcast(u16), magic_t[:, 0:ww], acc[:].bitcast(u16)
            )
            p = small.tile([128, ww], bf16, tag="p" + sfx, name=f"p{sfx}_{i}")
            nc.vector.tensor_mul(p[:], acc[:], r0[:])
            u = small.tile([128, ww], bf16, tag="u" + sfx, name=f"u{sfx}_{i}")
            nc.vector.scalar_tensor_tensor(
                out=u[:], in0=p[:], scalar=2.0, in1=r0[:],
                op0=ALU.subtract, op1=ALU.mult,
            )
            w_t = small.tile([128, ww], bf16, tag="w" + sfx, name=f"w{sfx}_{i}")
            nc.vector.scalar_tensor_tensor(
                out=w_t[:], in0=lab_ts[n][:, mc, lo:hi], scalar=BIAS_FIX,
                op0=ALU.mult, op1=ALU.mult, in1=u[:],
                accum_out=partial[:, pslot : pslot + 1],
            )

        HALF = W // 2
        LAST = NCH - 1

        def emit_exp(k):
            if k == LAST:      # halves: exp_a unblocks sum_a while exp_b runs
                stage_exp(k, 0, HALF)
                stage_exp(k, HALF, W)
            else:
                stage_exp(k)

        def emit_maxsub(k):
            if k == LAST:
                stage_maxsub(k, 0, HALF, "h")
                stage_maxsub(k, HALF, W, "h")
            else:
                stage_maxsub(k)

        def emit_sum(k):
            if k == LAST:
                stage_sum(k, 0, HALF, "h", pslot=k)
                stage_sum(k, HALF, W, "h", pslot=NCH)
            else:
                stage_sum(k)

        for i in range(NCH + 3):
            if i < NCH:
                n = i // MC
                if i % MC == 1 and n + 1 < n_loc:
                    load_first(n + 1)
                if i % MC == 2 and n + 1 < n_loc:
                    load_rest(n + 1)
                stage_mm(i)
            # exp AFTER the drains in the ACT queue: PE's psum-ring reuse
            # waits on drains, which must not queue behind a 3.8us exp
            if 0 <= i - 2 < NCH:
                emit_exp(i - 2)
            if 0 <= i - 1 < NCH:
                emit_maxsub(i - 1)
            if 0 <= i - 3 < NCH:
                emit_sum(i - 3)

        pf = singles.tile([128, 1], f32)
        nc.vector.tensor_reduce(
            pf[:], partial[:], axis=mybir.AxisListType.X, op=ALU.add
        )
        nc.sync.dma_start(out_d[:, :], pf[:])

    nc.compile()
    return nc


def _get_compiled():
    global _COMPILED
    if _COMPILED is None:
        _COMPILED = _build()
    return _COMPILED


def _host_prep(x, centers, labels):
    x = np.asarray(x, dtype=np.float32)
    centers = np.asarray(centers, dtype=np.float32)
    labels_np = np.asarray(labels)

    n_zero = int((labels_np == 0).sum())

    # sta[n, mc]: [128, NSTA, 128]; slot 3c+kc at [p, q] =
    #   -2*x[n, c, mc*128+q, kc*128+p]; slot 33 = eye(128)
    xt2 = np.transpose(x, (0, 1, 3, 2)) * -2.0          # [N, C, W(k), H]
    xt2 = xt2.reshape(N, C, KC, 128, MC, 128).astype(_FP8)
    sta = np.empty((N, MC, 128, NSTA, 128), dtype=_FP8)
    # -> [N, MC(h), 128(p=k), C, KC, 128(q=h)]
    sta[:, :, :, : C * KC, :] = np.transpose(xt2, (0, 4, 3, 1, 2, 5)).reshape(
        N, MC, 128, C * KC, 128
    )
    sta[:, :, :, C * KC, :] = np.eye(128, dtype=_FP8)[None, None]

    ee = (x * x + centers * centers).astype(_FP8).reshape(N, C, MC, 128, W)
    cc8 = centers.astype(_FP8).reshape(N, C, KC, 128, W)

    mva = np.empty((N, C, 4, 128, W), dtype=_FP8)
    mva[:, :, 0:3] = cc8
    mva[:, :, 3] = ee[:, :, 0]
    mva = np.ascontiguousarray(
        np.transpose(mva, (0, 3, 1, 2, 4)).reshape(N, 128, NMOVA * W)
    )
    mvb = np.empty((N, C, 3, 128, W), dtype=_FP8)
    mvb[:, :, 0] = cc8[:, :, 2]
    mvb[:, :, 1] = ee[:, :, 1]
    mvb[:, :, 2] = ee[:, :, 2]
    mvb = np.ascontiguousarray(
        np.transpose(mvb, (0, 3, 1, 2, 4)).reshape(N, 128, NMOVB * W)
    )

    lab = np.ascontiguousarray(
        np.transpose(
            labels_np.astype(np.float32).reshape(N, MC, 128, W), (0, 2, 1, 3)
        ).reshape(N, 128, MC * W)
    ).astype(_BF16)

    sta = sta.reshape(N, MC, 128, NSTA * 128)
    cst = np.zeros((128, 2, W), dtype=np.uint16)
    cst[:, 0, :] = MAGIC

    in_maps = []
    for core in range(N_CORES):
        sl = slice(core * N_LOC, (core + 1) * N_LOC)
        in_maps.append(
            {
                "sta": np.ascontiguousarray(sta[sl]),
                "mva": np.ascontiguousarray(mva[sl]),
                "mvb": np.ascontiguousarray(mvb[sl]),
                "lab": np.ascontiguousarray(lab[sl]),
                "cst": cst,
            }
        )
    return in_maps, n_zero


def kernel(x, centers, labels, _trace=False, _trace_kwargs=None):
    from concourse import bass_utils

    nc = _get_compiled()
    in_maps, n_zero = _host_prep(x, centers, labels)

    kwargs = {}
    if _trace:
        kwargs = dict(trace=True, **(_trace_kwargs or {}))
    res = bass_utils.run_bass_kernel_spmd(
        nc, in_maps, core_ids=list(range(N_CORES)), **kwargs
    )

    total = 0.0
    for core in range(N_CORES):
        total += float(res.results[core]["out"].astype(np.float64).sum())
    loss = (total + 1e-12 * n_zero) / float(N * H * W)
    out = np.float32(loss)
    if _trace:
        return out, res
    return out
